# revision 14
# baseline (speedup 1.0000x reference)
"""DecoupledBottleneckAttention on 8 trn2 NeuronCores.

Sharding: core c -> batch b=c//4, head-group g=c%4 (4 heads/core).
Each core computes q/k/v projections for its heads, causal attention,
and a partial out-projection; the host sums the 4 partials per batch.

v2 design (vs baseline):
- All matmul inputs bf16 (rel-err budget 2e-2; bf16 lands ~3e-3).
- Single pass over xT: qk and v projections share one staged x chunk.
- ob-outer projection loop: each PSUM bank accumulates its 16 dt
  matmuls then evicts while the next bank's matmuls stream -> PE never
  waits on evictions (the baseline stalled ~25us per s-chunk and HAM
  re-throttled to half clock 28% of the time).
- RoPE on full 128-partition tiles: heads are paired per weight block
  (sem block [h0|h1], geo block [h1|h0]) so evictions stay
  partition-aligned and DVE ops run all 128 lanes; the rotate-half
  sign is folded into the sin table.
- Softmax denominator via ones[128,128] matmul -> every PSUM partition
  holds the column sum, so normalization is copy + full-width
  reciprocal + one multiply (baseline: [1,512] reciprocal at 3.3us a
  pop plus a broadcast matmul + copy).
- Attention (q-chunk outer) interleaved with the out-projection of the
  finished chunk; exp() skips max-subtraction (logits bounded by the
  fixed input scale).
"""

import json
from contextlib import ExitStack

import numpy as np
import ml_dtypes

import jax
import concourse.bass as bass
import concourse.mybir as mybir
from concourse.tile import TileContext
from concourse import bass2jax
from concourse.bass2jax import Mesh, PartitionSpec, shard_map, partition_id_tensor

F32 = mybir.dt.float32
BF16 = mybir.dt.bfloat16
NPBF = ml_dtypes.bfloat16

B, S, D = 2, 2048, 2048
H = 16
HPC = 4  # heads per core
N_CORES = 8
DH = 128  # per-head v dim (q/k: 64 sem + 64 geo)
ROPE_BASE = 10000.0
SCALE = 1.0 / np.sqrt(128.0)

NSC = S // 512  # 4 s-chunks of 512
NDT = D // 128  # 16 contraction tiles
NST = S // 128  # 16 s-tiles of 128


def _split_multi_waits(bir: dict) -> dict:
    """walrus here rejects >1 sync waits per instruction; split extras
    into single-wait Drains inserted just before, on the same engine."""
    for fn in bir.get("functions", []):
        for blk in fn.get("blocks", []):
            new_insts = []
            for ins in blk.get("instructions", []):
                si = ins.get("sync_info") or {}
                waits = si.get("on_wait") or []
                if len(waits) > 1:
                    for i, w in enumerate(waits[:-1]):
                        new_insts.append(
                            {
                                "debug": ins.get("debug", 0),
                                "engine": ins["engine"],
                                "ins": [],
                                "name": f"{ins['name']}-w{i}",
                                "opcode": "Drain",
                                "outs": [],
                                "sync_info": {"on_update": [], "on_wait": [w]},
                            }
                        )
                    si["on_wait"] = [waits[-1]]
                new_insts.append(ins)
            blk["instructions"] = new_insts
    return bir


class _PatchedBass(bass.Bass):
    def to_json_bytes(self) -> bytes:
        return json.dumps(_split_multi_waits(json.loads(super().to_json_bytes()))).encode()


def _act_reciprocal(nc, out, in_):
    """Reciprocal on the scalar engine (~5x faster than DVE reciprocal,
    can read PSUM). bass guards ACT Reciprocal off for accuracy, but it
    measures 1.2e-5 max rel err on hardware here -- far inside this
    kernel's 2e-2 budget."""
    eng = nc.scalar
    ins = [eng.lower_ap(in_)]
    for v in (0.0, 1.0, 0.0):  # bias, scale, alpha
        ins.append(mybir.ImmediateValue(dtype=mybir.dt.float32, value=v))
    return eng.add_instruction(mybir.InstActivation(
        name=nc.get_next_instruction_name(),
        func=mybir.ActivationFunctionType.Reciprocal,
        ins=ins, outs=[eng.lower_ap(out)]))


def _build():
    nc = _PatchedBass("TRN2", target_bir_lowering=False, debug=False, num_devices=N_CORES)

    xT_d = nc.dram_tensor("xT", [D, S], BF16, kind="ExternalInput")
    wqk_d = nc.dram_tensor("wqk", [D, 8 * 128], BF16, kind="ExternalInput")
    wv_d = nc.dram_tensor("wv", [D, HPC * DH], BF16, kind="ExternalInput")
    wo_d = nc.dram_tensor("wo", [HPC * DH, D], BF16, kind="ExternalInput")
    # cols 0:S cos, S:2S signed sin (-sin on x1 rows, +sin on x2 rows),
    # 32-row blocks replicated 4x along partitions.
    cs_d = nc.dram_tensor("cs", [128, 2 * S], BF16, kind="ExternalInput")
    mask_d = nc.dram_tensor("mask", [128, 4 * 512], BF16, kind="ExternalInput")
    ones_d = nc.dram_tensor("ones", [128, 128], BF16, kind="ExternalInput")
    yp_d = nc.dram_tensor("yp", [S, D], BF16, kind="ExternalOutput")

    with TileContext(nc) as tc, ExitStack() as ctx, \
         nc.allow_low_precision(reason="bf16 compute; tolerance is 2e-2"):
        pers = ctx.enter_context(tc.tile_pool(name="pers", bufs=1))
        # per-head qT/kT [128 dims, S]; even heads [sem|geo], odd [geo|sem]
        qT = [pers.tile([128, S], BF16, name=f"qT{j}", tag=f"qT{j}") for j in range(HPC)]
        kT = [pers.tile([128, S], BF16, name=f"kT{j}", tag=f"kT{j}") for j in range(HPC)]
        v_sb = [pers.tile([128, HPC * DH], BF16, name=f"v{st}", tag=f"v{st}")
                for st in range(NST)]
        outT = [pers.tile([128, S], BF16, name=f"outT{j}", tag=f"outT{j}")
                for j in range(HPC)]
        cs_sb = pers.tile([128, 2 * S], BF16, name="cs_sb", tag="cs_sb")
        mask_sb = pers.tile([128, 4 * 512], BF16, name="mask_sb", tag="mask_sb")
        ones_sb = pers.tile([128, 128], BF16, name="ones_sb", tag="ones_sb")
        wqk_sb = [pers.tile([128, 8 * 128], BF16, name=f"wqk{dt}", tag=f"wqk{dt}")
                  for dt in range(NDT)]
        wv_sb = [pers.tile([128, HPC * DH], BF16, name=f"wv{dt}", tag=f"wv{dt}")
                 for dt in range(NDT)]
        wo_sb = [pers.tile([128, D], BF16, name=f"wo{j}", tag=f"wo{j}")
                 for j in range(HPC)]
        # ---- Phase AB: q/k/v projections, one x pass, RoPE on q/k geo ----
        xT_r = xT_d.rearrange("(dt p) s -> p dt s", p=128)
        with tc.tile_pool(name="xt", bufs=2) as xt_pool, \
             tc.tile_pool(name="rope", bufs=2) as rope_pool, \
             tc.tile_pool(name="psqk", bufs=4, space="PSUM") as psQK, \
             tc.tile_pool(name="psv", bufs=2, space="PSUM") as psV:
            # DMA preamble ordered first-needed-first, with the first x
            # chunk split per dt so the ob=0 matmul stream starts as soon
            # as wqk[0] + the first x slice land instead of after ~8MB.
            xt0 = xt_pool.tile([128, NDT * 512], BF16, name="xt", tag="xt")
            for dt in range(NDT):
                nc.sync.dma_start(out=wqk_sb[dt], in_=wqk_d[dt * 128:(dt + 1) * 128, :])
                nc.sync.dma_start(out=xt0[:, dt * 512:(dt + 1) * 512],
                                  in_=xT_d[dt * 128:(dt + 1) * 128, 0:512])
            nc.sync.dma_start(out=cs_sb, in_=cs_d[:, :])
            nc.sync.dma_start(out=mask_sb, in_=mask_d[:, :])
            for dt in range(NDT):
                nc.sync.dma_start(out=wv_sb[dt], in_=wv_d[dt * 128:(dt + 1) * 128, :])
            nc.sync.dma_start(out=ones_sb, in_=ones_d[:, :])
            for j in range(HPC):
                nc.sync.dma_start(out=wo_sb[j], in_=wo_d[j * 128:(j + 1) * 128, :])
            for sc in range(NSC):
                cols = slice(sc * 512, (sc + 1) * 512)
                if sc == 0:
                    xt = xt0
                else:
                    xt = xt_pool.tile([128, NDT * 512], BF16, name="xt", tag="xt")
                    nc.sync.dma_start(
                        out=xt.rearrange("p (dt c) -> p dt c", dt=NDT),
                        in_=xT_r[:, :, cols])
                csc = cs_sb[:, sc * 512:(sc + 1) * 512]
                sns = cs_sb[:, S + sc * 512:S + (sc + 1) * 512]
                for ob in range(8):
                    ps = psQK.tile([128, 512], F32, name="ps_qk", tag="qk")
                    for dt in range(NDT):
                        nc.tensor.matmul(
                            ps,
                            lhsT=wqk_sb[dt][:, ob * 128:(ob + 1) * 128],
                            rhs=xt[:, dt * 512:(dt + 1) * 512],
                            start=(dt == 0),
                            stop=(dt == NDT - 1),
                        )
                    dsts = qT if ob < 4 else kT
                    bi = ob % 4
                    he, ho = 2 * (bi % 2), 2 * (bi % 2) + 1  # head pair
                    if bi < 2:
                        # sem block [h_even | h_odd]
                        nc.scalar.activation(dsts[he][0:64, cols], ps[0:64, :],
                                             mybir.ActivationFunctionType.Copy)
                        nc.scalar.activation(dsts[ho][64:128, cols], ps[64:128, :],
                                             mybir.ActivationFunctionType.Copy)
                    else:
                        # geo block [h_odd | h_even], rotate-half RoPE.
                        # stage geo in SBUF (DMA cannot read PSUM), build the
                        # half-swapped copy, then full-width multiplies with
                        # the sign folded into the sin table.
                        stage = rope_pool.tile([128, 512], BF16, name="stage", tag="stage")
                        sw = rope_pool.tile([128, 512], BF16, name="sw", tag="sw")
                        tmp = rope_pool.tile([128, 512], BF16, name="tmp", tag="tmp")
                        tmp2 = rope_pool.tile([128, 512], BF16, name="tmp2", tag="tmp2")
                        nc.scalar.activation(stage, ps,
                                             mybir.ActivationFunctionType.Copy)
                        for gq in range(4):
                            a, b = gq * 32, (gq ^ 1) * 32
                            nc.sync.dma_start(out=sw[a:a + 32, :],
                                              in_=stage[b:b + 32, :])
                        nc.vector.tensor_mul(tmp, sw, sns)
                        nc.vector.tensor_mul(tmp2, stage, csc)
                        nc.vector.tensor_add(dsts[ho][0:64, cols],
                                             tmp2[0:64, :], tmp[0:64, :])
                        nc.vector.tensor_add(dsts[he][64:128, cols],
                                             tmp2[64:128, :], tmp[64:128, :])
                for st in range(4):
                    psv = psV.tile([128, HPC * DH], F32, name="ps_v", tag="v")
                    for dt in range(NDT):
                        nc.tensor.matmul(
                            psv,
                            lhsT=xt[:, dt * 512 + st * 128:dt * 512 + (st + 1) * 128],
                            rhs=wv_sb[dt],
                            start=(dt == 0),
                            stop=(dt == NDT - 1),
                        )
                    nc.scalar.activation(v_sb[sc * 4 + st], psv,
                                         mybir.ActivationFunctionType.Copy)

        # ---- Phase C+D: causal attention + out-projection per q-chunk ----
        with tc.tile_pool(name="attn", bufs=4) as attn_pool, \
             tc.tile_pool(name="lrec", bufs=2) as lrec_pool, \
             tc.tile_pool(name="ysb", bufs=3) as y_pool, \
             tc.tile_pool(name="psst", bufs=3, space="PSUM") as psST, \
             tc.tile_pool(name="psout", bufs=2, space="PSUM") as psOut, \
             tc.tile_pool(name="psl", bufs=1, space="PSUM") as psL, \
             tc.tile_pool(name="psd", bufs=2, space="PSUM") as psD:
            for qc in range(NSC):
                qcols = slice(qc * 512, (qc + 1) * 512)
                kmax = qc * 4 + 4
                for j in range(HPC):
                    outp = psOut.tile([128, 512], F32, name="outp", tag="outp")
                    lp = psL.tile([128, 512], F32, name="lp", tag="lp")
                    for kj in range(kmax):
                        st_ps = psST.tile([128, 512], F32, name="st_ps", tag="st")
                        nc.tensor.matmul(
                            st_ps,
                            lhsT=kT[j][:, kj * 128:(kj + 1) * 128],
                            rhs=qT[j][:, qcols],
                            start=True, stop=True,
                        )
                        p_sb = attn_pool.tile([128, 512], BF16, name="p_sb", tag="p")
                        nc.scalar.activation(p_sb, st_ps,
                                             mybir.ActivationFunctionType.Exp)
                        dj = kj - qc * 4
                        if dj >= 0:
                            nc.vector.tensor_mul(
                                p_sb, p_sb, mask_sb[:, dj * 512:(dj + 1) * 512])
                        nc.tensor.matmul(
                            outp,
                            lhsT=v_sb[kj][:, j * DH:(j + 1) * DH],
                            rhs=p_sb,
                            start=(kj == 0), stop=(kj == kmax - 1),
                        )
                        # ones[128,128] -> every partition of lp gets the
                        # column sum: broadcast denominator for free
                        nc.tensor.matmul(
                            lp,
                            lhsT=ones_sb,
                            rhs=p_sb,
                            start=(kj == 0), stop=(kj == kmax - 1),
                        )
                    rbc = lrec_pool.tile([128, 512], F32, name="rbc", tag="rbc")
                    _act_reciprocal(nc, rbc, lp)
                    nc.vector.tensor_mul(outT[j][:, qcols], outp, rbc)
                for st in range(4):
                    srow = qc * 4 + st
                    for mc in range(NSC):
                        yp_ps = psD.tile([128, 512], F32, name="yp_ps", tag="yd")
                        for j in range(HPC):
                            nc.tensor.matmul(
                                yp_ps,
                                lhsT=outT[j][:, srow * 128:(srow + 1) * 128],
                                rhs=wo_sb[j][:, mc * 512:(mc + 1) * 512],
                                start=(j == 0), stop=(j == HPC - 1),
                            )
                        # alternate eviction engine: exp() loads ACT, the
                        # normalize chain loads DVE -- split the difference
                        y_sb = y_pool.tile([128, 512], BF16, name="y_sb", tag="y")
                        if mc % 2 == 0:
                            nc.vector.tensor_scalar_mul(y_sb, yp_ps, 1.0)
                        else:
                            nc.scalar.activation(y_sb, yp_ps,
                                                 mybir.ActivationFunctionType.Copy)
                        nc.sync.dma_start(
                            out=yp_d[srow * 128:(srow + 1) * 128,
                                     mc * 512:(mc + 1) * 512],
                            in_=y_sb)
    return nc


class SpmdRunner:
    def __init__(self, nc, n_cores: int):
        bass2jax.install_neuronx_cc_hook()
        self.nc = nc
        self.n_cores = n_cores
        partition_name = nc.partition_id_tensor.name if nc.partition_id_tensor else None

        in_names, out_names, out_avals = [], [], []
        for alloc in nc.m.functions[0].allocations:
            if not isinstance(alloc, mybir.MemoryLocationSet):
                continue
            name = alloc.memorylocations[0].name
            if alloc.kind == "ExternalInput":
                if name != partition_name:
                    in_names.append(name)
            elif alloc.kind == "ExternalOutput":
                out_names.append(name)
                shape = tuple(alloc.tensor_shape)
                dtype = mybir.dt.np(alloc.dtype)
                out_avals.append(jax.core.ShapedArray(shape, dtype))
        self.in_names = list(in_names)
        self.out_names = out_names
        self.out_avals = out_avals
        n_params = len(in_names)
        all_in_names = in_names + out_names
        if partition_name is not None:
            all_in_names.append(partition_name)

        def _body(*args):
            operands = list(args)
            if partition_name is not None:
                operands.append(partition_id_tensor())
            outs = bass2jax._bass_exec_p.bind(
                *operands,
                out_avals=tuple(out_avals),
                in_names=tuple(all_in_names),
                out_names=tuple(out_names),
                lowering_input_output_aliases=(),
                sim_require_finite=True,
                sim_require_nnan=True,
                nc=nc,
            )
            return tuple(outs)

        devices = jax.devices()[:n_cores]
        self.mesh = Mesh(np.asarray(devices), ("core",))
        in_specs = (PartitionSpec("core"),) * (n_params + len(out_names))
        out_specs = (PartitionSpec("core"),) * len(out_names)
        donate = tuple(range(n_params, n_params + len(out_names)))
        self.jitted = jax.jit(
            shard_map(_body, mesh=self.mesh, in_specs=in_specs,
                      out_specs=out_specs, check_rep=False),
            donate_argnums=donate,
            keep_unused=True,
        )
        self.sharding = jax.sharding.NamedSharding(self.mesh, PartitionSpec("core"))
        # on-device zero allocator for the donated output buffers
        zero_shapes = [(n_cores * av.shape[0], *av.shape[1:]) for av in out_avals]
        zero_dtypes = [av.dtype for av in out_avals]

        def _mk_zeros():
            import jax.numpy as jnp
            return tuple(jnp.zeros(s, d) for s, d in zip(zero_shapes, zero_dtypes))

        self._mk_zeros = jax.jit(_mk_zeros, out_shardings=(self.sharding,) * len(out_avals))

    def concat_inputs(self, in_maps):
        assert len(in_maps) == self.n_cores
        return [
            np.concatenate([np.asarray(in_maps[c][n]) for c in range(self.n_cores)], axis=0)
            for n in self.in_names
        ]

    def stage(self, in_maps):
        arrs = self.concat_inputs(in_maps)
        staged = [jax.device_put(a, self.sharding) for a in arrs]
        jax.block_until_ready(staged)
        return staged

    def run_staged(self, staged):
        # no block between the two dispatches: they pipeline server-side,
        # so the wall cost is one tunnel round-trip + device time
        zeros = self._mk_zeros()
        outs = self.jitted(*staged, *zeros)
        jax.block_until_ready(outs)
        return outs

    def __call__(self, in_maps):
        staged = self.stage(in_maps)
        outs = self.run_staged(staged)
        res = []
        for c in range(self.n_cores):
            res.append({
                name: np.asarray(outs[i]).reshape(self.n_cores, *self.out_avals[i].shape)[c]
                for i, name in enumerate(self.out_names)
            })
        return res


_NC_CACHE: dict = {}


def _get_runner():
    if "runner" not in _NC_CACHE:
        _NC_CACHE["runner"] = SpmdRunner(_build(), N_CORES)
    return _NC_CACHE["runner"]


def _host_inputs(x, Wq_sem, Wk_sem, Wq_geo, Wk_geo, Wv, Wo):
    # RoPE tables: 32-row freq blocks replicated 4x; sin sign-folded
    # (-sin on x1 rows, +sin on x2 rows)
    inv_freq = 1.0 / (ROPE_BASE ** (np.arange(0, 64, 2, dtype=np.float32) / 64.0))
    t = np.arange(S, dtype=np.float32)
    freqs = np.outer(t, inv_freq)  # [S, 32]
    cosT = np.cos(freqs).T.astype(np.float32)  # [32, S]
    sinT = np.sin(freqs).T.astype(np.float32)
    cs = np.zeros((128, 2 * S), np.float32)
    for gq in range(4):
        r = slice(gq * 32, (gq + 1) * 32)
        cs[r, :S] = cosT
        cs[r, S:] = sinT if gq % 2 else -sinT

    # causal mask: mask[kl, dj*512 + ql] = ql >= dj*128 + kl
    ql = np.arange(512)
    kl = np.arange(128)
    mask = np.zeros((128, 4 * 512), np.float32)
    for dj in range(4):
        mask[:, dj * 512:(dj + 1) * 512] = (ql[None, :] >= dj * 128 + kl[:, None])

    ones = np.ones((128, 128), np.float32)

    in_maps = []
    for c in range(N_CORES):
        b, g = divmod(c, 4)
        # weight blocks: sem pair [h_even|h_odd], geo pair [h_odd|h_even]
        blocks = []
        for Wsem, Wgeo, sc_ in ((Wq_sem, Wq_geo, SCALE), (Wk_sem, Wk_geo, 1.0)):
            for pair in range(2):
                he, ho = g * HPC + 2 * pair, g * HPC + 2 * pair + 1
                blocks.append(np.concatenate(
                    [Wsem[he * 64:(he + 1) * 64], Wsem[ho * 64:(ho + 1) * 64]],
                    axis=0) * sc_)
            for pair in range(2):
                he, ho = g * HPC + 2 * pair, g * HPC + 2 * pair + 1
                blocks.append(np.concatenate(
                    [Wgeo[ho * 64:(ho + 1) * 64], Wgeo[he * 64:(he + 1) * 64]],
                    axis=0) * sc_)
        wqk = np.ascontiguousarray(np.concatenate(blocks, axis=0).T)
        hv = slice(g * HPC * DH, (g + 1) * HPC * DH)
        wv = np.ascontiguousarray(Wv[hv].T)
        wo = np.ascontiguousarray(Wo[:, hv].T)
        xT = np.ascontiguousarray(x[b].T)
        in_maps.append({
            "xT": xT.astype(NPBF),
            "wqk": wqk.astype(NPBF),
            "wv": wv.astype(NPBF),
            "wo": wo.astype(NPBF),
            "cs": cs.astype(NPBF),
            "mask": mask.astype(NPBF),
            "ones": ones.astype(NPBF),
        })
    return in_maps


def kernel(x, Wq_sem, Wk_sem, Wq_geo, Wk_geo, Wv, Wo):
    in_maps = _host_inputs(np.asarray(x), np.asarray(Wq_sem), np.asarray(Wk_sem),
                           np.asarray(Wq_geo), np.asarray(Wk_geo),
                           np.asarray(Wv), np.asarray(Wo))
    res = _get_runner()(in_maps)
    y = np.empty((B, S, D), np.float32)
    for b in range(B):
        y[b] = sum(res[b * 4 + g]["yp"].astype(np.float32) for g in range(4))
    return y
